# revision 1
# baseline (speedup 1.0000x reference)
"""Trainium2 Bass kernel for the DefenceWrapper sampling module.

Per row (batch=32768, C=1000 classes):
  raw = logits/6; mc = max(softmax(raw)); std = 0.3 + 0.6*mc^2
  noisy = raw + noise*std; p = softmax(noisy); p = clip(p, 0, 0.6)
  p /= sum(p); p = round(p*10)/10; if sum(p)==0: p = 1/C
  idx = inverse-CDF sample with threshold u*cumsum(p)[-1]
  out = log(one_hot(idx)*(1-eps) + eps/C)

The output has exactly two distinct f32 values:
  A = log(eps/C)          (cold)   bits 0xc180f1dc
  B = log(1-eps+eps/C)    (hot)    bits 0xb8d182ae
and is constructed bit-exactly via   out = min((iota == idx)*1e30 + A, B).

Sharding: pure data parallel, 4096 rows per core across 8 cores.
Row tiles of 128 (rows on partitions, classes along free dim); DMA moves
256 rows per transfer (1 MB) when pairing is enabled.

Numerical scheme:
  - unstabilized softmax (|raw| <= ~5 so exp is safe); max_conf via
    monotonicity: max(softmax) = max(e)/sum(e)
  - exp + row-sum fused in one ACT pass (activation accum_out)
  - round-half-even via the +2^23 magic constant (f32 RNE)
  - the rounded probs are scanned in x1000 units, where every value is a
    small exact f32 integer (uniform rows contribute exactly 1.0), so the
    cumsum is exact, tighter than any rounded f32 cumsum ordering
  - idx = fused accum of the (cum < u*cum[-1]) compare

Engine balance (measured on HW): ACT does only the two exp passes;
everything else runs on DVE. GPSIMD ops measured ~10x slower than the
cost model's guess, so it only does constant setup.
"""

import numpy as np

N_CORES = 8
C = 1000
P = 128

A_F = float(np.array([0xC180F1DC], dtype=np.uint32).view(np.float32)[0])
B_F = float(np.array([0xB8D182AE], dtype=np.uint32).view(np.float32)[0])
MAGIC = 8388608.0  # 2^23: x + MAGIC - MAGIC == RNE-round(x) for 0 <= x < 2^22
SQRT36 = float(np.float32(np.sqrt(np.float64(3.6))))
INV_T = 1.0 / 6.0
POS_BIG = 1e30

# Engine placement config (ablation knob): values "dve" | "gps" | "act".
CFG = {
    "smalls": "dve",   # q, std6, s3d, ua, th
    "rf": "dve",
    "oh": "dve",
    "out": "dve",
    "magic": "dve",    # act | dve
    "unmagic": "dve",  # act | dve
    "e2_bf16": True,
    "pair": True,      # load/store 256 rows per DMA (1 MB transfers)
    "skip": set(),     # timing experiments only (breaks correctness)
}


def build_sampler(tc, out_ap, logits_ap, noise_ap, u_ap, repeat=1):
    """Emit the sampling pipeline into TileContext `tc`.

    APs are DRAM access patterns: out/logits/noise are [rows, C] f32,
    u is [rows, 1] f32. rows must be a multiple of 128.

    repeat > 1 wraps the whole tile loop in a hardware For_i that redoes
    the identical (idempotent) work; used only for wall-clock benchmarking.
    """
    from contextlib import ExitStack, nullcontext

    from concourse import mybir

    nc = tc.nc
    rows = logits_ap.shape[0]
    assert rows % P == 0
    ntiles = rows // P

    f32 = mybir.dt.float32
    bf16 = mybir.dt.bfloat16
    i32 = mybir.dt.int32

    with ExitStack() as ctx:
        const = ctx.enter_context(tc.tile_pool(name="const", bufs=1))
        big = ctx.enter_context(tc.tile_pool(name="big", bufs=3))
        work = ctx.enter_context(tc.tile_pool(name="work", bufs=4))
        small = ctx.enter_context(tc.tile_pool(name="small", bufs=5))

        # Constants: per-row u thresholds, f32 iota along classes, 0.6 clip.
        u_sb = const.tile([P, ntiles], f32, tag="u")
        nc.sync.dma_start(
            out=u_sb[:], in_=u_ap.flatten().rearrange("(t p) -> p t", p=P)
        )
        iota_i = const.tile([P, C], i32, tag="iota_i")
        nc.gpsimd.iota(iota_i[:], pattern=[[1, C]], base=0, channel_multiplier=0)
        iota_f = const.tile([P, C], f32, tag="iota_f")
        nc.vector.tensor_copy(iota_f[:], iota_i[:])
        c06 = const.tile([P, C], bf16, tag="c06")
        nc.gpsimd.memset(c06[:], 0.6)
        negmagic = const.tile([P, 1], f32, tag="negmagic")
        nc.gpsimd.memset(negmagic[:], -MAGIC)

        rep_ctx = tc.For_i(0, repeat, 1) if repeat > 1 else nullcontext()
        with rep_ctx:
            _emit_tiles(
                nc, big, work, small, out_ap, logits_ap, noise_ap,
                u_sb, iota_f, c06, negmagic, ntiles, mybir,
            )


def _emit_tiles(
    nc, big, work, small, out_ap, logits_ap, noise_ap,
    u_sb, iota_f, c06, negmagic, ntiles, mybir,
):
    Exp = mybir.ActivationFunctionType.Exp
    Copy = mybir.ActivationFunctionType.Copy
    Ident = mybir.ActivationFunctionType.Identity
    Op = mybir.AluOpType
    X = mybir.AxisListType.X
    f32 = mybir.dt.float32
    bf16 = mybir.dt.bfloat16

    sm_eng = nc.gpsimd if CFG["smalls"] == "gps" else nc.vector
    rf_eng = nc.gpsimd if CFG["rf"] == "gps" else nc.vector
    oh_eng = nc.gpsimd if CFG["oh"] == "gps" else nc.vector
    out_eng = nc.gpsimd if CFG["out"] == "gps" else nc.vector
    e2_dt = bf16 if CFG["e2_bf16"] else f32
    skip = CFG["skip"]

    pair = CFG["pair"] and ntiles % 2 == 0
    G = 2 if pair else 1

    def dram3(ap, t0):
        v = ap[t0 * P : (t0 + G) * P, :]
        return v.rearrange("(a p) c -> p a c", p=P) if pair else v

    if "compute" in skip:
        # DMA-floor measurement: load both inputs, copy one out on ACT.
        for t in range(0, ntiles, G):
            lg = big.tile([P, G, C], f32, tag="lg")
            nc.sync.dma_start(out=lg[:], in_=dram3(logits_ap, t))
            nz = big.tile([P, G, C], f32, tag="nz")
            nc.sync.dma_start(out=nz[:], in_=dram3(noise_ap, t))
            out = big.tile([P, G, C], f32, tag="out")
            nc.scalar.activation(out[:], lg[:], Copy, bias=0.0, scale=1.0)
            nc.sync.dma_start(out=dram3(out_ap, t), in_=out[:])
        return

    for tp in range(0, ntiles, G):
        lg2 = big.tile([P, G, C], f32, tag="lg")
        nc.sync.dma_start(out=lg2[:], in_=dram3(logits_ap, tp))
        nz2 = big.tile([P, G, C], f32, tag="nz")
        nc.sync.dma_start(out=nz2[:], in_=dram3(noise_ap, tp))
        out2 = big.tile([P, G, C], f32, tag="out")

        for h in range(G):
            t = tp + h
            lg = lg2[:, h] if pair else lg2[:]
            nz = nz2[:, h] if pair else nz2[:]
            outh = out2[:, h] if pair else out2[:]

            if "max" in skip:
                std6 = small.tile([P, 1], f32, tag="std6")
                nc.vector.memset(std6[:], 1.8)
            else:
                # e1 = exp(logits/6), s1 = row-sum(e1) in one ACT pass
                e1 = work.tile([P, C], f32, tag="e1")
                s1 = small.tile([P, 1], f32, tag="s1")
                nc.scalar.activation(
                    e1[:], lg, Exp, scale=INV_T, accum_out=s1[:]
                )

                # max_conf = max(e1)/s1 ; std6 = 3.6*mc^2 + 1.8
                me = small.tile([P, 1], f32, tag="me")
                nc.vector.tensor_reduce(me[:], e1[:], axis=X, op=Op.max)
                rs1 = small.tile([P, 1], f32, tag="rs1")
                nc.vector.reciprocal(rs1[:], s1[:])
                q = small.tile([P, 1], f32, tag="q")
                sm_eng.tensor_scalar(
                    q[:], me[:], rs1[:], SQRT36, Op.mult, Op.mult
                )
                std6 = small.tile([P, 1], f32, tag="std6")
                sm_eng.tensor_scalar(std6[:], q[:], q[:], 1.8, Op.mult, Op.add)

            # noisy*6 = noise*std6 + logits ; e2 = exp(noisy6/6), s2 = sum
            ny = work.tile([P, C], f32, tag="ny")
            nc.vector.scalar_tensor_tensor(
                ny[:], nz, std6[:], lg, Op.mult, Op.add
            )
            e2 = work.tile([P, C], e2_dt, tag="e2")
            s2 = small.tile([P, 1], f32, tag="s2")
            nc.scalar.activation(e2[:], ny[:], Exp, scale=INV_T, accum_out=s2[:])

            # probs = e2/s2 clipped at 0.6; s3 = row-sum of clipped.
            # All-bf16 operands keep the DVE in its 2x mode.
            rs2 = small.tile([P, 1], f32, tag="rs2")
            nc.vector.reciprocal(rs2[:], s2[:])
            pc = work.tile([P, C], e2_dt, tag="pc")
            s3 = small.tile([P, 1], f32, tag="s3")
            nc.vector.scalar_tensor_tensor(
                pc[:], e2[:], rs2[:], c06[:], Op.mult, Op.min, accum_out=s3[:]
            )

            # R10 = round(pc*(10/s3)) via the 2^23 RNE trick;
            # rsum10 = sum(R10) (row is all-zero iff rsum10 == 0)
            s3d = small.tile([P, 1], f32, tag="s3d")
            sm_eng.tensor_scalar(s3d[:], s3[:], 0.1, None, Op.mult, Op.bypass)
            sc10 = small.tile([P, 1], f32, tag="sc10")
            nc.vector.reciprocal(sc10[:], s3d[:])
            m = work.tile([P, C], f32, tag="m")
            if CFG["magic"] == "act":
                nc.scalar.activation(m[:], pc[:], Copy, bias=MAGIC, scale=sc10[:])
            else:
                nc.vector.tensor_scalar(
                    m[:], pc[:], sc10[:], MAGIC, Op.mult, Op.add
                )
            r10 = work.tile([P, C], f32, tag="r10")
            rsum10 = small.tile([P, 1], f32, tag="rsum10")
            if CFG["unmagic"] == "act":
                nc.scalar.activation(
                    r10[:], m[:], Copy, bias=-MAGIC, scale=1.0,
                    accum_out=rsum10[:],
                )
            else:
                nc.vector.tensor_scalar(
                    r10[:], m[:], MAGIC, None, Op.subtract, Op.add,
                    accum_out=rsum10[:],
                )

            # Scan units are x1000: rf = R10*100 (+1.0 on all-zero rows, the
            # uniform 1/C case) — small exact f32 integers.
            ua = small.tile([P, 1], f32, tag="ua")
            sm_eng.tensor_scalar(
                ua[:], rsum10[:], 0.0, None, Op.is_equal, Op.bypass
            )
            rf = work.tile([P, C], f32, tag="rf")
            rf_eng.tensor_scalar(rf[:], r10[:], 100.0, ua[:], Op.mult, Op.add)

            # exact integer cumsum; thresh = u * cum[-1]
            cum = work.tile([P, C], f32, tag="cum")
            if "scan" in skip:
                nc.vector.tensor_scalar(
                    cum[:], rf[:], 1.0, None, Op.mult, Op.bypass
                )
            else:
                nc.vector.tensor_tensor_scan(
                    cum[:], rf[:], rf[:], 0.0, Op.add, Op.bypass
                )
            th = small.tile([P, 1], f32, tag="th")
            sm_eng.tensor_scalar(
                th[:], cum[:, C - 1 : C], u_sb[:, t : t + 1], None,
                Op.mult, Op.bypass,
            )

            # idx = #(cum < thresh) via fused accum of the compare
            s = work.tile([P, C], f32, tag="oh")
            idx = small.tile([P, 1], f32, tag="idx")
            nc.vector.tensor_scalar(
                s[:], cum[:], th[:], None, Op.is_lt, Op.add, accum_out=idx[:]
            )

            # out = min((iota == idx)*1e30 + A, B): bit-exact A/B everywhere
            oh = work.tile([P, C], f32, tag="oh")
            oh_eng.tensor_scalar(
                oh[:], iota_f[:], idx[:], POS_BIG, Op.is_equal, Op.mult
            )
            out_eng.tensor_scalar(outh, oh[:], A_F, B_F, Op.add, Op.min)

        nc.sync.dma_start(out=dram3(out_ap, tp), in_=out2[:])


_NC_CACHE = {}


def _get_nc(rows_per_core):
    if rows_per_core in _NC_CACHE:
        return _NC_CACHE[rows_per_core]
    from concourse import bacc, mybir
    from concourse.tile import TileContext

    nc = bacc.Bacc(
        "TRN2",
        target_bir_lowering=False,
        debug=False,
        enable_asserts=False,
        num_devices=N_CORES,
    )
    logits_d = nc.dram_tensor(
        "logits", [rows_per_core, C], mybir.dt.float32, kind="ExternalInput"
    )
    noise_d = nc.dram_tensor(
        "noise", [rows_per_core, C], mybir.dt.float32, kind="ExternalInput"
    )
    u_d = nc.dram_tensor(
        "u", [rows_per_core, 1], mybir.dt.float32, kind="ExternalInput"
    )
    out_d = nc.dram_tensor(
        "out", [rows_per_core, C], mybir.dt.float32, kind="ExternalOutput"
    )
    with TileContext(nc) as tc:
        build_sampler(tc, out_d.ap(), logits_d.ap(), noise_d.ap(), u_d.ap())
    nc.compile()
    _NC_CACHE[rows_per_core] = nc
    return nc


def kernel(logits, noise, u, _trace=False):
    from concourse.bass_utils import run_bass_kernel_spmd

    logits = np.ascontiguousarray(logits, dtype=np.float32)
    noise = np.ascontiguousarray(noise, dtype=np.float32)
    u = np.ascontiguousarray(u, dtype=np.float32)
    batch = logits.shape[0]
    assert batch % N_CORES == 0
    rows = batch // N_CORES
    nc = _get_nc(rows)
    in_maps = [
        {
            "logits": logits[i * rows : (i + 1) * rows],
            "noise": noise[i * rows : (i + 1) * rows],
            "u": u[i * rows : (i + 1) * rows],
        }
        for i in range(N_CORES)
    ]
    res = run_bass_kernel_spmd(
        nc, in_maps, list(range(N_CORES)), trace=_trace
    )
    out = np.concatenate([res.results[i]["out"] for i in range(N_CORES)], axis=0)
    if _trace:
        return out, res
    return out

